# revision 32
# baseline (speedup 1.0000x reference)
"""Causal self-attention (RoPE) Trainium2 Bass kernel (v2).

Sharding: 8 cores = 2 (batch) x 4 (head groups). Each core computes one batch
element b and 4 of the 16 heads end-to-end (QKV projection -> RoPE -> causal
attention -> c_proj rows), producing a partial [T, C] output; the host sums
the 4 partials per batch element (the "all-reduce" of the row-sharded c_proj).

v2 layout tricks:
- Q/K are computed head-major with lo/hi RoPE halves INTERLEAVED pairwise
  (row 2d = dim d, row 2d+1 = dim d+32 of a head). rotate_half is then a
  within-32-partition swap = one DVE stream_shuffle, and each head occupies
  64 contiguous partitions so the scores matmul contracts 64 rows in ONE
  instruction (PE cost is per output column, independent of contraction
  depth). The host permutes W_attn's Q/K columns to emit this layout.
- bf16 operands everywhere on the PE (1 cycle/col at any width).
- Diagonal 128-col blocks restrict exp/AV to the unmasked query range.
- attention is processed in 2 head-pair groups so PSUM holds double-buffered
  score tiles (pipelining) alongside the AV accumulators.
- qkv-projection and c_proj units of neighboring tiles are interleaved into
  the attention kb loop so the PE fills the exp-latency gaps.
"""

import os
import sys
import numpy as np

N_CORES = 8
B, T, C = 2, 2048, 1024
H = 16
HD = 64
HPC = 4            # heads per core
NT = 4             # token tiles of 512
TQ = 512           # tq tile size
KC = C // 128      # contraction chunks for qkv projection

_PROGRAM_CACHE = {}

# stream_shuffle mask: swap even/odd partitions within each 32-block
SWAP_EVEN_ODD = [i ^ 1 for i in range(32)]


def _build_program(has_battn: bool, has_bproj: bool, debug_taps: bool = False):
    import concourse.bass as bass
    import concourse.mybir as mybir
    import concourse.bacc as bacc
    import concourse.tile as tile

    F32 = mybir.dt.float32
    F32R = mybir.dt.float32r
    BF16 = mybir.dt.bfloat16

    nc = bacc.Bacc("TRN2", target_bir_lowering=False, debug=False,
                   num_devices=N_CORES)

    dbg = {}
    if debug_taps:
        for name, shape in [("dbg_q0", [128, T]), ("dbg_k0", [128, T]),
                            ("dbg_vaug", [128, 260]), ("dbg_s2", [128, 1024]),
                            ("dbg_pt", [128, 1024]), ("dbg_yr", [65, 512]),
                            ("dbg_y0", [128, T])]:
            dbg[name] = nc.dram_tensor(name, shape, F32,
                                       kind="ExternalOutput").ap()

    xT = nc.dram_tensor("xT", [C, T], BF16, kind="ExternalInput").ap()
    wqkv = nc.dram_tensor("wqkv", [C, 768], BF16, kind="ExternalInput").ap()
    cos_il = nc.dram_tensor("cos_il", [128, T], BF16, kind="ExternalInput").ap()
    sin_il = nc.dram_tensor("sin_il", [128, T], BF16, kind="ExternalInput").ap()
    wp = nc.dram_tensor("wp", [2 * 128, C], BF16, kind="ExternalInput").ap()
    battn = (nc.dram_tensor("battn", [1, 768], BF16, kind="ExternalInput").ap()
             if has_battn else None)
    bproj = (nc.dram_tensor("bproj", [1, C], F32R, kind="ExternalInput").ap()
             if has_bproj else None)
    out = nc.dram_tensor("out", [T, C], F32, kind="ExternalOutput").ap()
    # DRAM scratch for the 1/sumexp partition-broadcast bounce
    rec_d = nc.dram_tensor("rec_d", [16, 512], F32)

    Exp = mybir.ActivationFunctionType.Exp
    scale = 1.0 / float(np.sqrt(HD))

    with tile.TileContext(nc) as tc:
        with (
            tc.tile_pool(name="const", bufs=1) as const,
            tc.tile_pool(name="xp", bufs=1) as xp,
            tc.tile_pool(name="qk", bufs=1) as qkp,
            tc.tile_pool(name="vaug", bufs=1) as vaugp,
            tc.tile_pool(name="tmp", bufs=4) as tmp,
            tc.tile_pool(name="shp", bufs=3) as shp,
            tc.tile_pool(name="pp", bufs=4) as pp,
            tc.tile_pool(name="rp", bufs=4) as rp,
            tc.tile_pool(name="yrp", bufs=4) as yrp,
            tc.tile_pool(name="yp", bufs=1) as yp,
            tc.tile_pool(name="op", bufs=3) as op,
            tc.tile_pool(name="ps_qv", bufs=2, space="PSUM") as ps_qv,
            tc.tile_pool(name="ps_s", bufs=2, space="PSUM") as ps_s,
            tc.tile_pool(name="ps_av", bufs=2, space="PSUM") as ps_av,
        ):
            # ---- constants.  Split only enough for queue parallelism; every
            # dma_start costs ~600ns of serial SP-engine issue time, so
            # fine-grained splitting backfires.
            w_kc = []
            xt_view = xT.rearrange("(kc p) t -> p kc t", p=128)
            xt0 = xp.tile([128, KC, TQ], BF16, tag="xt", name="xt_0")
            wq_view = wqkv.rearrange("(kc p) f -> p kc f", p=128)
            for kc in range(KC):
                w = const.tile([128, 768], BF16, tag=f"w{kc}")
                if kc == 0:
                    # halve the latency of the two chunks gating matmul 0
                    for p2 in range(2):
                        psl = slice(64 * p2, 64 * p2 + 64)
                        nc.sync.dma_start(w[psl, 0:512],
                                          wq_view[psl, kc, 0:512])
                        nc.sync.dma_start(xt0[psl, kc, :],
                                          xt_view[psl, kc, 0:TQ])
                else:
                    nc.sync.dma_start(w[:, 0:512], wq_view[:, kc, 0:512])
                    nc.sync.dma_start(xt0[:, kc, :], xt_view[:, kc, 0:TQ])
                nc.sync.dma_start(w[:, 512:768], wq_view[:, kc, 512:768])
                w_kc.append(w)
            cos_sb = const.tile([128, T], BF16, tag="cos")
            sin_sb = const.tile([128, T], BF16, tag="sin")
            for h4 in range(2):
                sl = slice(h4 * 1024, (h4 + 1) * 1024)
                nc.sync.dma_start(cos_sb[:, sl], cos_il[:, sl])
                nc.sync.dma_start(sin_sb[:, sl], sin_il[:, sl])
            wp_sb = const.tile([128, 2, C], BF16, tag="wp")
            wp_view = wp.rearrange("(kb p) f -> p kb f", p=128)
            for kb in range(2):
                nc.sync.dma_start(wp_sb[:, kb, :], wp_view[:, kb, :])
            ones_f = const.tile([1, 512], F32, tag="ones_f")
            nc.vector.memset(ones_f[:], 1.0)
            ones_r = const.tile([1, 512], F32R, tag="ones_r")
            nc.vector.tensor_copy(ones_r[:], ones_f[:])
            if has_battn:
                ones_b = const.tile([1, 512], BF16, tag="ones_b")
                nc.vector.tensor_copy(ones_b[:], ones_f[:])
            if has_battn:
                battn_sb = const.tile([1, 768], BF16, tag="battn")
                nc.sync.dma_start(battn_sb[:], battn[:])
            if has_bproj:
                bproj_sb = const.tile([1, C], F32R, tag="bproj")
                nc.sync.dma_start(bproj_sb[:], bproj[:])

            # persistent activations
            # q/k tiles: heads (2g, 2g+1) in rows [0:64],[64:128], lo/hi dims
            # interleaved pairwise within each head
            q_t = [qkp.tile([128, T], BF16, tag=f"q{g}", name=f"q_{g}")
                   for g in range(2)]
            k_t = [qkp.tile([128, T], BF16, tag=f"k{g}", name=f"k_{g}")
                   for g in range(2)]
            v_aug = vaugp.tile([128, 16, HPC * 65], BF16, tag="vaug")
            y0 = yp.tile([128, T], BF16, tag="y0")
            y1 = yp.tile([128, T], BF16, tag="y1")

            # ones columns of v_aug (col 64 of each head's 65-wide slot)
            for tb in range(16):
                va = v_aug[:, tb, :].rearrange("p (h c) -> p h c", c=65)
                nc.vector.memset(va[:, :, 64:65], 1.0)

            # prefetch remaining x tiles
            xts = [xt0]
            for n in range(1, NT):
                tsl = bass.ts(n, TQ)
                xt = xp.tile([128, KC, TQ], BF16, tag="xt", name=f"xt_{n}")
                for kc in range(KC):
                    nc.sync.dma_start(xt[:, kc, :], xt_view[:, kc, tsl])
                xts.append(xt)

            # ---------------- qkv projection units -----------------
            def qk_block(n, blk):
                """project q/k block blk (0=q01,1=q23,2=k01,3=k23) of tile n
                and apply rope."""
                tsl = bass.ts(n, TQ)
                xt = xts[n]
                pq = ps_qv.tile([128, TQ], F32, tag="pqv",
                                name=f"pq_{n}_{blk}")
                for kc in range(KC):
                    nc.tensor.matmul(
                        pq[:], w_kc[kc][:, bass.ts(blk, 128)], xt[:, kc, :],
                        start=(kc == 0),
                        stop=(kc == KC - 1 and not has_battn))
                if has_battn:
                    nc.tensor.matmul(
                        pq[:], battn_sb[0:1, bass.ts(blk, 128)],
                        ones_b[0:1, :TQ], start=False, stop=True)
                # rope: out = p*cos + swap_pairs(p)*sin_signed
                p_s = shp.tile([128, TQ], BF16, tag="ps")
                nc.vector.tensor_copy(p_s[:], pq[:])
                sh = shp.tile([128, TQ], BF16, tag="sh")
                nc.vector.stream_shuffle(sh[:], p_s[:], mask=SWAP_EVEN_ODD)
                t1 = tmp.tile([128, TQ], BF16, tag="t")
                nc.vector.tensor_mul(t1[:], p_s[:], cos_sb[:, tsl])
                t2 = tmp.tile([128, TQ], BF16, tag="t")
                nc.gpsimd.tensor_mul(t2[:], sh[:], sin_sb[:, tsl])
                dst = (q_t[0], q_t[1], k_t[0], k_t[1])[blk]
                nc.vector.tensor_add(dst[:, tsl], t1[:], t2[:])

            def v_block(n, tb):
                """project v for token block 4n+tb into v_aug."""
                blk = 4 * n + tb
                xt = xts[n]
                pv = ps_qv.tile([128, TQ], F32, tag="pqv",
                                name=f"pv_{n}_{tb}")
                for kc in range(KC):
                    nc.tensor.matmul(
                        pv[:, 0:256], xt[:, kc, bass.ts(tb, 128)],
                        w_kc[kc][:, 512:768],
                        start=(kc == 0),
                        stop=(kc == KC - 1 and not has_battn))
                if has_battn:
                    nc.tensor.matmul(
                        pv[:, 0:256], ones_b[0:1, 0:128],
                        battn_sb[0:1, 512:768], start=False, stop=True)
                va = v_aug[:, blk, :].rearrange("p (h c) -> p h c", c=65)
                pv_h = pv[:, 0:256].rearrange("p (h d) -> p h d", d=64)
                nc.vector.tensor_copy(va[:, :, 0:64], pv_h[:])

            def qkv_units(n):
                return ([lambda n=n, b=b_: qk_block(n, b) for b_ in range(4)]
                        + [lambda n=n, t=t_: v_block(n, t) for t_ in range(4)])

            def cproj_unit(i, m):
                """c_proj + store for token block m (128 tokens)."""
                msl = bass.ts(m, 128)
                o_t = op.tile([128, C], F32, tag="o", name=f"o_{m}")
                for n2 in range(2):
                    nsl = bass.ts(n2, 512)
                    po = ps_qv.tile([128, TQ], F32, tag="pqv",
                                    name=f"po_{m}_{n2}")
                    nc.tensor.matmul(po[:], y0[:, msl], wp_sb[:, 0, nsl],
                                     start=True, stop=False)
                    nc.tensor.matmul(po[:], y1[:, msl], wp_sb[:, 1, nsl],
                                     start=False, stop=not has_bproj)
                    if has_bproj:
                        nc.tensor.matmul(po[:], ones_r[0:1, 0:128],
                                         bproj_sb[0:1, nsl],
                                         start=False, stop=True)
                    # alternate psum eviction between DVE and ACT to keep
                    # the vector queue shallow
                    if (m + n2) % 2 == 0:
                        nc.vector.tensor_copy(o_t[:, nsl], po[:])
                    else:
                        nc.scalar.copy(o_t[:, nsl], po[:])
                    nc.sync.dma_start(out[msl, nsl], o_t[:, nsl])

            def cproj_units(i):
                return [lambda i=i, m=m_: cproj_unit(i, m)
                        for m_ in range(4 * i, 4 * i + 4)]

            # ---------------- attention -----------------
            def norm_unit(i, grp, half, yr, rec_r):
                """deferred normalize: broadcast 1/sumexp via a ones-column
                matmul (the DVE part of the chain was issued at eviction
                time, so the PE never stalls on it here)."""
                h = 2 * grp + half
                rb = ps_qv.tile([64, TQ], F32, tag="pqv", name=f"rb_{i}_{h}")
                nc.tensor.matmul(rb[:], ones_r[0:1, 0:64], rec_r[:],
                                 start=True, stop=True)
                y_tile = y0 if grp == 0 else y1
                y_sl = y_tile[64 * half:64 * half + 64, bass.ts(i, TQ)]
                nc.vector.tensor_mul(y_sl, yr[0:64, :], rb[:])

            def attn_tile(i, fillers, norm_out):
                """attention for query tile i; pops filler units (qkv of
                tile i+1 / cproj of tile i-1 / deferred normalizes) between
                kb iterations.  grp-1 normalize units are appended to
                norm_out for the next tile."""
                tq_sl = bass.ts(i, TQ)
                n_k = 4 * i + 4
                n_iters = 2 * n_k
                it = 0
                for grp in range(2):
                    kt = k_t[grp]
                    qt = q_t[grp]
                    av = [ps_av.tile([65, TQ], F32, tag="av",
                                     name=f"av_{i}_{grp}_{h}")
                          for h in range(2)]
                    for kb in range(n_k):
                        ksl = bass.ts(kb, 128)
                        j = kb - 4 * i
                        skip = 128 * j if j > 0 else 0
                        W_v = TQ - skip
                        s2 = ps_s.tile([128, 2 * TQ], F32, tag="s2",
                                       name=f"s_{i}_{grp}_{kb}")
                        for half in range(2):
                            dst = s2[:, half * TQ + skip:(half + 1) * TQ]
                            qsl = bass.ds(i * TQ + skip, W_v)
                            hsl = slice(64 * half, 64 * half + 64)
                            nc.tensor.matmul(dst, kt[hsl, ksl], qt[hsl, qsl],
                                             start=True, stop=True,
                                             tile_position=(64 * half, 0))
                        p_t = pp.tile([128, 2, TQ], BF16, tag="p")
                        s2_v = s2[:].rearrange("p (g t) -> p g t", g=2)
                        if debug_taps and i == 1 and grp == 0 and kb == 2:
                            d_s2 = tmp.tile([128, 1024], F32, tag="dbg",
                                            name="d_s2")
                            nc.vector.tensor_copy(d_s2[:], s2[:])
                            nc.sync.dma_start(dbg["dbg_s2"][:], d_s2[:])
                        nc.scalar.activation(p_t[:, :, skip:TQ],
                                             s2_v[:, :, skip:TQ],
                                             Exp, scale=scale)
                        if j >= 0:
                            # 128-wide causal triangle (keep local col >= row)
                            for half in range(2):
                                nc.gpsimd.affine_select(
                                    out=p_t[:, half, skip:skip + 128],
                                    in_=p_t[:, half, skip:skip + 128],
                                    compare_op=mybir.AluOpType.is_ge,
                                    fill=0.0,
                                    base=0,
                                    pattern=[[1, 128]],
                                    channel_multiplier=-1,
                                )
                        for half in range(2):
                            h = 2 * grp + half
                            nc.tensor.matmul(
                                av[half][:, skip:TQ],
                                v_aug[:, kb, bass.ts(h, 65)],
                                p_t[:, half, skip:TQ],
                                start=(kb == 0),
                                stop=(kb == n_k - 1),
                            )
                        if debug_taps and i == 1 and grp == 0 and kb == 2:
                            d_pt = tmp.tile([128, 1024], F32, tag="dbg",
                                            name="d_pt")
                            nc.vector.tensor_copy(
                                d_pt[:].rearrange("p (g t) -> p g t", g=2),
                                p_t[:])
                            nc.sync.dma_start(dbg["dbg_pt"][:], d_pt[:])
                        it += 1
                        # pop fillers whose target iteration has arrived
                        while fillers and fillers[0][0] <= it:
                            _, fn = fillers.pop(0)
                            fn()

                    # evict the two finished heads; issue the DVE reciprocal
                    # chain now (runs async), defer the PE broadcast + y mul
                    # so the in-order PE queue never waits on it.
                    for half in range(2):
                        h = 2 * grp + half
                        yr = yrp.tile([65, TQ], F32, tag="yr")
                        nc.vector.tensor_copy(yr[:], av[half][:])
                        if debug_taps and i == 1 and h == 0:
                            nc.sync.dma_start(dbg["dbg_yr"][:], yr[:])
                        se = rp.tile([1, TQ], F32, tag="se")
                        nc.vector.tensor_copy(se[:], yr[64:65, :])
                        rec = rp.tile([1, TQ], F32, tag="r")
                        nc.vector.reciprocal_approx_fast(rec[:], se[:])
                        rec_r = rp.tile([1, TQ], F32R, tag="rr")
                        nc.vector.tensor_copy(rec_r[:], rec[:])
                        nu = (lambda i=i, g=grp, hf=half, y=yr, r=rec_r:
                              norm_unit(i, g, hf, y, r))
                        if grp == 0:
                            # run during grp 1's kb loop of this tile
                            fillers.append((it + 2 + half, nu))
                            fillers.sort(key=lambda p: p[0])
                        else:
                            norm_out.append(nu)

            # ---------------- schedule -----------------
            # tile 0 qkv runs up front; tile n+1 qkv, tile n-1 cproj, and
            # deferred normalizes interleave into tile n's attention kb loop.
            for u in qkv_units(0):
                u()
            norm_prev = []
            for i in range(NT):
                base = list(norm_prev)  # must issue before cproj(i-1)
                norm_prev = []
                if i + 1 < NT:
                    base += qkv_units(i + 1)
                if i >= 1:
                    base += cproj_units(i - 1)
                n_iters = 2 * (4 * i + 4)
                n_f = len(base)
                fillers = [(min(n_iters, 1 + (fi * n_iters) // max(n_f, 1)),
                            fn) for fi, fn in enumerate(base)]
                attn_tile(i, fillers, norm_prev)
                for _, fn in fillers:
                    fn()
            for fn in norm_prev:
                fn()
            for u in cproj_units(NT - 1):
                u()

            if debug_taps:
                for nm, src in (("dbg_q0", q_t[0]), ("dbg_k0", k_t[0]),
                                ("dbg_y0", y0)):
                    for half in range(4):
                        hsl = bass.ts(half, TQ)
                        d_t = tmp.tile([128, TQ], F32, tag="dbg",
                                       name=f"d_{nm}_{half}")
                        nc.vector.tensor_copy(d_t[:], src[:, hsl])
                        nc.sync.dma_start(dbg[nm][:, hsl], d_t[:])
                d_v = tmp.tile([128, 260], F32, tag="dbgv", name="d_v")
                nc.vector.tensor_copy(d_v[:], v_aug[:, 3, :])
                nc.sync.dma_start(dbg["dbg_vaug"][:], d_v[:])

    nc.finalize()
    return nc


def _get_debug_program(has_battn, has_bproj):
    key = (has_battn, has_bproj, "dbg")
    if key not in _PROGRAM_CACHE:
        _PROGRAM_CACHE[key] = _build_program(has_battn, has_bproj,
                                             debug_taps=True)
    return _PROGRAM_CACHE[key]


def _get_program(has_battn, has_bproj):
    key = (has_battn, has_bproj)
    if key not in _PROGRAM_CACHE:
        _PROGRAM_CACHE[key] = _build_program(*key)
    return _PROGRAM_CACHE[key]


def _rope_tables_np():
    """cos/sin tables in interleaved-pair layout, sign folded into sin.

    Row 2d   of a 64-block: lo dim d  -> cos(f_d), -sin(f_d)
    Row 2d+1 of a 64-block: hi dim d  -> cos(f_d), +sin(f_d)
    """
    inv_freq = (1.0 / (10000.0 ** (np.arange(0, HD, 2, dtype=np.float32) / HD)))
    t = np.arange(T, dtype=np.float32)
    freqs = np.outer(inv_freq, t).astype(np.float32)      # [32, T]
    cos = np.cos(freqs)
    sin = np.sin(freqs)
    cos64 = np.empty((64, T), dtype=np.float32)
    sin64 = np.empty((64, T), dtype=np.float32)
    cos64[0::2] = cos
    cos64[1::2] = cos
    sin64[0::2] = -sin
    sin64[1::2] = sin
    return (np.ascontiguousarray(np.tile(cos64, (2, 1))),
            np.ascontiguousarray(np.tile(sin64, (2, 1))))


def _install_trace_shim():
    """Optional: lets run_bass_kernel_spmd(trace=True) capture NTFF profiles."""
    import contextlib
    import ctypes
    import types

    so = "/opt/axon/libaxon_pjrt.so"
    if not os.path.exists(so) or "antenv.axon_hooks" in sys.modules:
        return
    try:
        lib = ctypes.CDLL(so)
        if not hasattr(lib, "axon_start_nrt_profile"):
            return
        lib.axon_start_nrt_profile.argtypes = [ctypes.POINTER(ctypes.c_int64),
                                               ctypes.c_size_t]
        lib.axon_start_nrt_profile.restype = ctypes.c_int64
        lib.axon_stop_nrt_profile.argtypes = [ctypes.c_char_p]
        lib.axon_stop_nrt_profile.restype = ctypes.c_int64

        @contextlib.contextmanager
        def _hook(output_dir, device_ids):
            import jax
            jax.devices()
            if device_ids:
                ids = (ctypes.c_int64 * len(device_ids))(*device_ids)
                rc = lib.axon_start_nrt_profile(ids, len(device_ids))
            else:
                rc = lib.axon_start_nrt_profile(None, 0)
            if rc != 0:
                raise RuntimeError(f"axon_start_nrt_profile rc={rc}")
            try:
                yield
            finally:
                n = lib.axon_stop_nrt_profile(str(output_dir).encode())
                print(f"profile: {n} file(s) written to {output_dir}",
                      file=sys.stderr)

        mod = types.ModuleType("antenv.axon_hooks")
        mod.get_axon_ntff_profile_hook = lambda: _hook
        mod.set_axon_ntff_profile_hook = lambda h: None
        sys.modules["antenv.axon_hooks"] = mod
    except Exception:
        pass


def _to_bf16(a):
    import ml_dtypes
    return np.ascontiguousarray(a.astype(ml_dtypes.bfloat16))


def kernel(x, W_attn, b_attn, W_proj, b_proj):
    from concourse.bass_utils import run_bass_kernel_spmd

    x = np.asarray(x, dtype=np.float32)
    W_attn = np.asarray(W_attn, dtype=np.float32)
    b_attn = np.asarray(b_attn, dtype=np.float32)
    W_proj = np.asarray(W_proj, dtype=np.float32)
    b_proj = np.asarray(b_proj, dtype=np.float32)

    has_battn = bool(np.any(b_attn))
    has_bproj = bool(np.any(b_proj))
    if os.environ.get("BASSK_DEBUG"):
        nc = _get_debug_program(has_battn, has_bproj)
    else:
        nc = _get_program(has_battn, has_bproj)

    cos_il, sin_il = _rope_tables_np()
    dd64 = np.arange(64)
    # interleaved lo/hi order within a head: [0,32,1,33,...,31,63]
    il = np.empty(64, dtype=np.int64)
    il[0::2] = np.arange(32)
    il[1::2] = np.arange(32) + 32

    in_maps = []
    for c in range(N_CORES):
        b = c // 4
        g = c % 4
        hs = 4 * g + np.arange(HPC)
        qcols = (hs[:, None] * HD + il[None, :]).ravel()   # interleaved
        vcols = (hs[:, None] * HD + dd64[None, :]).ravel()  # natural
        cols = np.concatenate([qcols, 1024 + qcols, 2048 + vcols])
        rows = vcols
        m = {
            "xT": _to_bf16(x[b].T),
            "wqkv": _to_bf16(W_attn[:, cols]),
            "cos_il": _to_bf16(cos_il),
            "sin_il": _to_bf16(sin_il),
            "wp": _to_bf16(W_proj[rows, :]),
        }
        if has_battn:
            m["battn"] = _to_bf16(b_attn[cols][None, :])
        if has_bproj:
            bp = b_proj if g == 0 else np.zeros_like(b_proj)
            m["bproj"] = np.ascontiguousarray(bp[None, :])
        in_maps.append(m)

    trace_dir = os.environ.get("BASSK_TRACE")
    kwargs = {}
    if trace_dir:
        _install_trace_shim()
        kwargs = {"trace": True, "tmpdir": trace_dir,
                  "trace_cores": [0], "stitch_traces": False}

    res = run_bass_kernel_spmd(nc, in_maps, core_ids=list(range(N_CORES)),
                               **kwargs)
    global _LAST_RES
    _LAST_RES = res
    if trace_dir:
        kernel._last_result = res

    out = np.zeros((B, T, C), dtype=np.float32)
    for c in range(N_CORES):
        out[c // 4] += res.results[c]["out"]
    return out


# revision 34
# speedup vs baseline: 1.0932x; 1.0932x over previous
"""Causal self-attention (RoPE) Trainium2 Bass kernel (v2).

Sharding: 8 cores = 2 (batch) x 4 (head groups). Each core computes one batch
element b and 4 of the 16 heads end-to-end (QKV projection -> RoPE -> causal
attention -> c_proj rows), producing a partial [T, C] output; the host sums
the 4 partials per batch element (the "all-reduce" of the row-sharded c_proj).

v2 layout tricks:
- Q/K are computed head-major with lo/hi RoPE halves INTERLEAVED pairwise
  (row 2d = dim d, row 2d+1 = dim d+32 of a head). rotate_half is then a
  within-32-partition swap = one DVE stream_shuffle, and each head occupies
  64 contiguous partitions so the scores matmul contracts 64 rows in ONE
  instruction (PE cost is per output column, independent of contraction
  depth). The host permutes W_attn's Q/K columns to emit this layout.
- bf16 operands everywhere on the PE (1 cycle/col at any width).
- Diagonal 128-col blocks restrict exp/AV to the unmasked query range.
- attention is processed in 2 head-pair groups so PSUM holds double-buffered
  score tiles (pipelining) alongside the AV accumulators.
- qkv-projection and c_proj units of neighboring tiles are interleaved into
  the attention kb loop so the PE fills the exp-latency gaps.
"""

import os
import sys
import numpy as np

N_CORES = 8
B, T, C = 2, 2048, 1024
H = 16
HD = 64
HPC = 4            # heads per core
NT = 4             # token tiles of 512
TQ = 512           # tq tile size
KC = C // 128      # contraction chunks for qkv projection

_PROGRAM_CACHE = {}

# stream_shuffle mask: swap even/odd partitions within each 32-block
SWAP_EVEN_ODD = [i ^ 1 for i in range(32)]


def _build_program(has_battn: bool, has_bproj: bool, debug_taps: bool = False):
    import concourse.bass as bass
    import concourse.mybir as mybir
    import concourse.bacc as bacc
    import concourse.tile as tile

    F32 = mybir.dt.float32
    F32R = mybir.dt.float32r
    BF16 = mybir.dt.bfloat16

    nc = bacc.Bacc("TRN2", target_bir_lowering=False, debug=False,
                   num_devices=N_CORES)

    dbg = {}
    if debug_taps:
        for name, shape in [("dbg_q0", [128, T]), ("dbg_k0", [128, T]),
                            ("dbg_vaug", [128, 260]), ("dbg_s2", [128, 1024]),
                            ("dbg_pt", [128, 1024]), ("dbg_yr", [65, 512]),
                            ("dbg_y0", [128, T])]:
            dbg[name] = nc.dram_tensor(name, shape, F32,
                                       kind="ExternalOutput").ap()

    xT = nc.dram_tensor("xT", [C, T], BF16, kind="ExternalInput").ap()
    wqkv = nc.dram_tensor("wqkv", [C, 768], BF16, kind="ExternalInput").ap()
    cos_il = nc.dram_tensor("cos_il", [128, T], BF16, kind="ExternalInput").ap()
    sin_il = nc.dram_tensor("sin_il", [128, T], BF16, kind="ExternalInput").ap()
    wp = nc.dram_tensor("wp", [2 * 128, C], BF16, kind="ExternalInput").ap()
    battn = (nc.dram_tensor("battn", [1, 768], BF16, kind="ExternalInput").ap()
             if has_battn else None)
    bproj = (nc.dram_tensor("bproj", [1, C], F32R, kind="ExternalInput").ap()
             if has_bproj else None)
    out = nc.dram_tensor("out", [T, C], F32, kind="ExternalOutput").ap()
    # DRAM scratch for the 1/sumexp partition-broadcast bounce
    rec_d = nc.dram_tensor("rec_d", [16, 512], F32)

    Exp = mybir.ActivationFunctionType.Exp
    scale = 1.0 / float(np.sqrt(HD))

    with tile.TileContext(nc) as tc:
        with (
            tc.tile_pool(name="const", bufs=1) as const,
            tc.tile_pool(name="xp", bufs=1) as xp,
            tc.tile_pool(name="qk", bufs=1) as qkp,
            tc.tile_pool(name="vaug", bufs=1) as vaugp,
            tc.tile_pool(name="tmp", bufs=4) as tmp,
            tc.tile_pool(name="shp", bufs=3) as shp,
            tc.tile_pool(name="pp", bufs=4) as pp,
            tc.tile_pool(name="rp", bufs=4) as rp,
            tc.tile_pool(name="yrp", bufs=4) as yrp,
            tc.tile_pool(name="yp", bufs=1) as yp,
            tc.tile_pool(name="op", bufs=3) as op,
            tc.tile_pool(name="ps_qv", bufs=2, space="PSUM") as ps_qv,
            tc.tile_pool(name="ps_s", bufs=2, space="PSUM") as ps_s,
            tc.tile_pool(name="ps_av", bufs=2, space="PSUM") as ps_av,
        ):
            # ---- constants.  Split only enough for queue parallelism; every
            # dma_start costs ~600ns of serial SP-engine issue time, so
            # fine-grained splitting backfires.
            w_kc = []
            xt_view = xT.rearrange("(kc p) t -> p kc t", p=128)
            xt0 = xp.tile([128, KC, TQ], BF16, tag="xt", name="xt_0")
            wq_view = wqkv.rearrange("(kc p) f -> p kc f", p=128)
            for kc in range(KC):
                w = const.tile([128, 768], BF16, tag=f"w{kc}")
                if kc == 0:
                    # halve the latency of the two chunks gating matmul 0
                    for p2 in range(2):
                        psl = slice(64 * p2, 64 * p2 + 64)
                        nc.sync.dma_start(w[psl, 0:512],
                                          wq_view[psl, kc, 0:512])
                        nc.sync.dma_start(xt0[psl, kc, :],
                                          xt_view[psl, kc, 0:TQ])
                else:
                    nc.sync.dma_start(w[:, 0:512], wq_view[:, kc, 0:512])
                    nc.sync.dma_start(xt0[:, kc, :], xt_view[:, kc, 0:TQ])
                nc.sync.dma_start(w[:, 512:768], wq_view[:, kc, 512:768])
                w_kc.append(w)
            cos_sb = const.tile([128, T], BF16, tag="cos")
            sin_sb = const.tile([128, T], BF16, tag="sin")
            for h4 in range(2):
                sl = slice(h4 * 1024, (h4 + 1) * 1024)
                nc.sync.dma_start(cos_sb[:, sl], cos_il[:, sl])
                nc.sync.dma_start(sin_sb[:, sl], sin_il[:, sl])
            wp_sb = const.tile([128, 2, C], BF16, tag="wp")
            wp_view = wp.rearrange("(kb p) f -> p kb f", p=128)
            for kb in range(2):
                nc.sync.dma_start(wp_sb[:, kb, :], wp_view[:, kb, :])
            ones_f = const.tile([1, 512], F32, tag="ones_f")
            nc.vector.memset(ones_f[:], 1.0)
            ones_r = const.tile([1, 512], F32R, tag="ones_r")
            nc.vector.tensor_copy(ones_r[:], ones_f[:])
            if has_battn:
                ones_b = const.tile([1, 512], BF16, tag="ones_b")
                nc.vector.tensor_copy(ones_b[:], ones_f[:])
            if has_battn:
                battn_sb = const.tile([1, 768], BF16, tag="battn")
                nc.sync.dma_start(battn_sb[:], battn[:])
            if has_bproj:
                bproj_sb = const.tile([1, C], F32R, tag="bproj")
                nc.sync.dma_start(bproj_sb[:], bproj[:])

            # persistent activations
            # q/k tiles: heads (2g, 2g+1) in rows [0:64],[64:128], lo/hi dims
            # interleaved pairwise within each head
            q_t = [qkp.tile([128, T], BF16, tag=f"q{g}", name=f"q_{g}")
                   for g in range(2)]
            k_t = [qkp.tile([128, T], BF16, tag=f"k{g}", name=f"k_{g}")
                   for g in range(2)]
            v_aug = vaugp.tile([128, 16, HPC * 65], BF16, tag="vaug")
            y0 = yp.tile([128, T], BF16, tag="y0")
            y1 = yp.tile([128, T], BF16, tag="y1")

            # ones columns of v_aug (col 64 of each head's 65-wide slot)
            for tb in range(16):
                va = v_aug[:, tb, :].rearrange("p (h c) -> p h c", c=65)
                nc.vector.memset(va[:, :, 64:65], 1.0)

            # prefetch remaining x tiles
            xts = [xt0]
            for n in range(1, NT):
                tsl = bass.ts(n, TQ)
                xt = xp.tile([128, KC, TQ], BF16, tag="xt", name=f"xt_{n}")
                for kc in range(KC):
                    nc.sync.dma_start(xt[:, kc, :], xt_view[:, kc, tsl])
                xts.append(xt)

            # ---------------- qkv projection units -----------------
            def qk_block(n, blk):
                """project q/k block blk (0=q01,1=q23,2=k01,3=k23) of tile n
                and apply rope."""
                tsl = bass.ts(n, TQ)
                xt = xts[n]
                pq = ps_qv.tile([128, TQ], F32, tag="pqv",
                                name=f"pq_{n}_{blk}")
                for kc in range(KC):
                    nc.tensor.matmul(
                        pq[:], w_kc[kc][:, bass.ts(blk, 128)], xt[:, kc, :],
                        start=(kc == 0),
                        stop=(kc == KC - 1 and not has_battn))
                if has_battn:
                    nc.tensor.matmul(
                        pq[:], battn_sb[0:1, bass.ts(blk, 128)],
                        ones_b[0:1, :TQ], start=False, stop=True)
                # rope: out = p*cos + swap_pairs(p)*sin_signed
                p_s = shp.tile([128, TQ], BF16, tag="ps")
                nc.vector.tensor_copy(p_s[:], pq[:])
                sh = shp.tile([128, TQ], BF16, tag="sh")
                nc.vector.stream_shuffle(sh[:], p_s[:], mask=SWAP_EVEN_ODD)
                t1 = tmp.tile([128, TQ], BF16, tag="t")
                nc.vector.tensor_mul(t1[:], p_s[:], cos_sb[:, tsl])
                t2 = tmp.tile([128, TQ], BF16, tag="t")
                nc.vector.tensor_mul(t2[:], sh[:], sin_sb[:, tsl])
                dst = (q_t[0], q_t[1], k_t[0], k_t[1])[blk]
                nc.vector.tensor_add(dst[:, tsl], t1[:], t2[:])

            def v_block(n, tb):
                """project v for token block 4n+tb into v_aug."""
                blk = 4 * n + tb
                xt = xts[n]
                pv = ps_qv.tile([128, TQ], F32, tag="pqv",
                                name=f"pv_{n}_{tb}")
                for kc in range(KC):
                    nc.tensor.matmul(
                        pv[:, 0:256], xt[:, kc, bass.ts(tb, 128)],
                        w_kc[kc][:, 512:768],
                        start=(kc == 0),
                        stop=(kc == KC - 1 and not has_battn))
                if has_battn:
                    nc.tensor.matmul(
                        pv[:, 0:256], ones_b[0:1, 0:128],
                        battn_sb[0:1, 512:768], start=False, stop=True)
                va = v_aug[:, blk, :].rearrange("p (h c) -> p h c", c=65)
                pv_h = pv[:, 0:256].rearrange("p (h d) -> p h d", d=64)
                nc.vector.tensor_copy(va[:, :, 0:64], pv_h[:])

            def qkv_units(n):
                return ([lambda n=n, b=b_: qk_block(n, b) for b_ in range(4)]
                        + [lambda n=n, t=t_: v_block(n, t) for t_ in range(4)])

            def cproj_unit(i, m):
                """c_proj + store for token block m (128 tokens)."""
                msl = bass.ts(m, 128)
                o_t = op.tile([128, C], F32, tag="o", name=f"o_{m}")
                for n2 in range(2):
                    nsl = bass.ts(n2, 512)
                    po = ps_qv.tile([128, TQ], F32, tag="pqv",
                                    name=f"po_{m}_{n2}")
                    nc.tensor.matmul(po[:], y0[:, msl], wp_sb[:, 0, nsl],
                                     start=True, stop=False)
                    nc.tensor.matmul(po[:], y1[:, msl], wp_sb[:, 1, nsl],
                                     start=False, stop=not has_bproj)
                    if has_bproj:
                        nc.tensor.matmul(po[:], ones_r[0:1, 0:128],
                                         bproj_sb[0:1, nsl],
                                         start=False, stop=True)
                    nc.vector.tensor_copy(o_t[:, nsl], po[:])
                    nc.sync.dma_start(out[msl, nsl], o_t[:, nsl])

            def cproj_units(i):
                return [lambda i=i, m=m_: cproj_unit(i, m)
                        for m_ in range(4 * i, 4 * i + 4)]

            # ---------------- attention -----------------
            def norm_unit(i, grp, half, yr, rec_r):
                """deferred normalize: broadcast 1/sumexp via a ones-column
                matmul (the DVE part of the chain was issued at eviction
                time, so the PE never stalls on it here)."""
                h = 2 * grp + half
                rb = ps_qv.tile([64, TQ], F32, tag="pqv", name=f"rb_{i}_{h}")
                nc.tensor.matmul(rb[:], ones_r[0:1, 0:64], rec_r[:],
                                 start=True, stop=True)
                y_tile = y0 if grp == 0 else y1
                y_sl = y_tile[64 * half:64 * half + 64, bass.ts(i, TQ)]
                nc.vector.tensor_mul(y_sl, yr[0:64, :], rb[:])

            def attn_tile(i, fillers, norm_out):
                """attention for query tile i; pops filler units (qkv of
                tile i+1 / cproj of tile i-1 / deferred normalizes) between
                kb iterations.  grp-1 normalize units are appended to
                norm_out for the next tile."""
                tq_sl = bass.ts(i, TQ)
                n_k = 4 * i + 4
                n_iters = 2 * n_k
                it = 0
                for grp in range(2):
                    kt = k_t[grp]
                    qt = q_t[grp]
                    av = [ps_av.tile([65, TQ], F32, tag="av",
                                     name=f"av_{i}_{grp}_{h}")
                          for h in range(2)]
                    for kb in range(n_k):
                        ksl = bass.ts(kb, 128)
                        j = kb - 4 * i
                        skip = 128 * j if j > 0 else 0
                        W_v = TQ - skip
                        s2 = ps_s.tile([128, 2 * TQ], F32, tag="s2",
                                       name=f"s_{i}_{grp}_{kb}")
                        for half in range(2):
                            dst = s2[:, half * TQ + skip:(half + 1) * TQ]
                            qsl = bass.ds(i * TQ + skip, W_v)
                            hsl = slice(64 * half, 64 * half + 64)
                            nc.tensor.matmul(dst, kt[hsl, ksl], qt[hsl, qsl],
                                             start=True, stop=True,
                                             tile_position=(64 * half, 0))
                        p_t = pp.tile([128, 2, TQ], BF16, tag="p")
                        s2_v = s2[:].rearrange("p (g t) -> p g t", g=2)
                        if debug_taps and i == 1 and grp == 0 and kb == 2:
                            d_s2 = tmp.tile([128, 1024], F32, tag="dbg",
                                            name="d_s2")
                            nc.vector.tensor_copy(d_s2[:], s2[:])
                            nc.sync.dma_start(dbg["dbg_s2"][:], d_s2[:])
                        nc.scalar.activation(p_t[:, :, skip:TQ],
                                             s2_v[:, :, skip:TQ],
                                             Exp, scale=scale)
                        if j >= 0:
                            # 128-wide causal triangle (keep local col >= row)
                            for half in range(2):
                                nc.gpsimd.affine_select(
                                    out=p_t[:, half, skip:skip + 128],
                                    in_=p_t[:, half, skip:skip + 128],
                                    compare_op=mybir.AluOpType.is_ge,
                                    fill=0.0,
                                    base=0,
                                    pattern=[[1, 128]],
                                    channel_multiplier=-1,
                                )
                        for half in range(2):
                            h = 2 * grp + half
                            nc.tensor.matmul(
                                av[half][:, skip:TQ],
                                v_aug[:, kb, bass.ts(h, 65)],
                                p_t[:, half, skip:TQ],
                                start=(kb == 0),
                                stop=(kb == n_k - 1),
                            )
                        if debug_taps and i == 1 and grp == 0 and kb == 2:
                            d_pt = tmp.tile([128, 1024], F32, tag="dbg",
                                            name="d_pt")
                            nc.vector.tensor_copy(
                                d_pt[:].rearrange("p (g t) -> p g t", g=2),
                                p_t[:])
                            nc.sync.dma_start(dbg["dbg_pt"][:], d_pt[:])
                        it += 1
                        # pop fillers whose target iteration has arrived
                        while fillers and fillers[0][0] <= it:
                            _, fn = fillers.pop(0)
                            fn()

                    # evict the two finished heads; issue the DVE reciprocal
                    # chain now (runs async), defer the PE broadcast + y mul
                    # so the in-order PE queue never waits on it.
                    for half in range(2):
                        h = 2 * grp + half
                        yr = yrp.tile([65, TQ], F32, tag="yr")
                        nc.vector.tensor_copy(yr[:], av[half][:])
                        if debug_taps and i == 1 and h == 0:
                            nc.sync.dma_start(dbg["dbg_yr"][:], yr[:])
                        se = rp.tile([1, TQ], F32, tag="se")
                        nc.vector.tensor_copy(se[:], yr[64:65, :])
                        rec = rp.tile([1, TQ], F32, tag="r")
                        nc.vector.reciprocal_approx_fast(rec[:], se[:])
                        rec_r = rp.tile([1, TQ], F32R, tag="rr")
                        nc.vector.tensor_copy(rec_r[:], rec[:])
                        nu = (lambda i=i, g=grp, hf=half, y=yr, r=rec_r:
                              norm_unit(i, g, hf, y, r))
                        if grp == 0:
                            # run during grp 1's kb loop of this tile
                            fillers.append((it + 2 + half, nu))
                            fillers.sort(key=lambda p: p[0])
                        else:
                            norm_out.append(nu)

            # ---------------- schedule -----------------
            # tile 0 qkv runs up front; tile n+1 qkv, tile n-1 cproj, and
            # deferred normalizes interleave into tile n's attention kb loop.
            for u in qkv_units(0):
                u()
            norm_prev = []
            for i in range(NT):
                base = list(norm_prev)  # must issue before cproj(i-1)
                norm_prev = []
                if i + 1 < NT:
                    base += qkv_units(i + 1)
                if i >= 1:
                    base += cproj_units(i - 1)
                n_iters = 2 * (4 * i + 4)
                n_f = len(base)
                fillers = [(min(n_iters, 1 + (fi * n_iters) // max(n_f, 1)),
                            fn) for fi, fn in enumerate(base)]
                attn_tile(i, fillers, norm_prev)
                for _, fn in fillers:
                    fn()
            for fn in norm_prev:
                fn()
            for u in cproj_units(NT - 1):
                u()

            if debug_taps:
                for nm, src in (("dbg_q0", q_t[0]), ("dbg_k0", k_t[0]),
                                ("dbg_y0", y0)):
                    for half in range(4):
                        hsl = bass.ts(half, TQ)
                        d_t = tmp.tile([128, TQ], F32, tag="dbg",
                                       name=f"d_{nm}_{half}")
                        nc.vector.tensor_copy(d_t[:], src[:, hsl])
                        nc.sync.dma_start(dbg[nm][:, hsl], d_t[:])
                d_v = tmp.tile([128, 260], F32, tag="dbgv", name="d_v")
                nc.vector.tensor_copy(d_v[:], v_aug[:, 3, :])
                nc.sync.dma_start(dbg["dbg_vaug"][:], d_v[:])

    nc.finalize()
    return nc


def _get_debug_program(has_battn, has_bproj):
    key = (has_battn, has_bproj, "dbg")
    if key not in _PROGRAM_CACHE:
        _PROGRAM_CACHE[key] = _build_program(has_battn, has_bproj,
                                             debug_taps=True)
    return _PROGRAM_CACHE[key]


def _get_program(has_battn, has_bproj):
    key = (has_battn, has_bproj)
    if key not in _PROGRAM_CACHE:
        _PROGRAM_CACHE[key] = _build_program(*key)
    return _PROGRAM_CACHE[key]


def _rope_tables_np():
    """cos/sin tables in interleaved-pair layout, sign folded into sin.

    Row 2d   of a 64-block: lo dim d  -> cos(f_d), -sin(f_d)
    Row 2d+1 of a 64-block: hi dim d  -> cos(f_d), +sin(f_d)
    """
    inv_freq = (1.0 / (10000.0 ** (np.arange(0, HD, 2, dtype=np.float32) / HD)))
    t = np.arange(T, dtype=np.float32)
    freqs = np.outer(inv_freq, t).astype(np.float32)      # [32, T]
    cos = np.cos(freqs)
    sin = np.sin(freqs)
    cos64 = np.empty((64, T), dtype=np.float32)
    sin64 = np.empty((64, T), dtype=np.float32)
    cos64[0::2] = cos
    cos64[1::2] = cos
    sin64[0::2] = -sin
    sin64[1::2] = sin
    return (np.ascontiguousarray(np.tile(cos64, (2, 1))),
            np.ascontiguousarray(np.tile(sin64, (2, 1))))


def _install_trace_shim():
    """Optional: lets run_bass_kernel_spmd(trace=True) capture NTFF profiles."""
    import contextlib
    import ctypes
    import types

    so = "/opt/axon/libaxon_pjrt.so"
    if not os.path.exists(so) or "antenv.axon_hooks" in sys.modules:
        return
    try:
        lib = ctypes.CDLL(so)
        if not hasattr(lib, "axon_start_nrt_profile"):
            return
        lib.axon_start_nrt_profile.argtypes = [ctypes.POINTER(ctypes.c_int64),
                                               ctypes.c_size_t]
        lib.axon_start_nrt_profile.restype = ctypes.c_int64
        lib.axon_stop_nrt_profile.argtypes = [ctypes.c_char_p]
        lib.axon_stop_nrt_profile.restype = ctypes.c_int64

        @contextlib.contextmanager
        def _hook(output_dir, device_ids):
            import jax
            jax.devices()
            if device_ids:
                ids = (ctypes.c_int64 * len(device_ids))(*device_ids)
                rc = lib.axon_start_nrt_profile(ids, len(device_ids))
            else:
                rc = lib.axon_start_nrt_profile(None, 0)
            if rc != 0:
                raise RuntimeError(f"axon_start_nrt_profile rc={rc}")
            try:
                yield
            finally:
                n = lib.axon_stop_nrt_profile(str(output_dir).encode())
                print(f"profile: {n} file(s) written to {output_dir}",
                      file=sys.stderr)

        mod = types.ModuleType("antenv.axon_hooks")
        mod.get_axon_ntff_profile_hook = lambda: _hook
        mod.set_axon_ntff_profile_hook = lambda h: None
        sys.modules["antenv.axon_hooks"] = mod
    except Exception:
        pass


def _to_bf16(a):
    import ml_dtypes
    return np.ascontiguousarray(a.astype(ml_dtypes.bfloat16))


def kernel(x, W_attn, b_attn, W_proj, b_proj):
    from concourse.bass_utils import run_bass_kernel_spmd

    x = np.asarray(x, dtype=np.float32)
    W_attn = np.asarray(W_attn, dtype=np.float32)
    b_attn = np.asarray(b_attn, dtype=np.float32)
    W_proj = np.asarray(W_proj, dtype=np.float32)
    b_proj = np.asarray(b_proj, dtype=np.float32)

    has_battn = bool(np.any(b_attn))
    has_bproj = bool(np.any(b_proj))
    if os.environ.get("BASSK_DEBUG"):
        nc = _get_debug_program(has_battn, has_bproj)
    else:
        nc = _get_program(has_battn, has_bproj)

    cos_il, sin_il = _rope_tables_np()
    dd64 = np.arange(64)
    # interleaved lo/hi order within a head: [0,32,1,33,...,31,63]
    il = np.empty(64, dtype=np.int64)
    il[0::2] = np.arange(32)
    il[1::2] = np.arange(32) + 32

    in_maps = []
    for c in range(N_CORES):
        b = c // 4
        g = c % 4
        hs = 4 * g + np.arange(HPC)
        qcols = (hs[:, None] * HD + il[None, :]).ravel()   # interleaved
        vcols = (hs[:, None] * HD + dd64[None, :]).ravel()  # natural
        cols = np.concatenate([qcols, 1024 + qcols, 2048 + vcols])
        rows = vcols
        m = {
            "xT": _to_bf16(x[b].T),
            "wqkv": _to_bf16(W_attn[:, cols]),
            "cos_il": _to_bf16(cos_il),
            "sin_il": _to_bf16(sin_il),
            "wp": _to_bf16(W_proj[rows, :]),
        }
        if has_battn:
            m["battn"] = _to_bf16(b_attn[cols][None, :])
        if has_bproj:
            bp = b_proj if g == 0 else np.zeros_like(b_proj)
            m["bproj"] = np.ascontiguousarray(bp[None, :])
        in_maps.append(m)

    trace_dir = os.environ.get("BASSK_TRACE")
    kwargs = {}
    if trace_dir:
        _install_trace_shim()
        kwargs = {"trace": True, "tmpdir": trace_dir,
                  "trace_cores": [0], "stitch_traces": False}

    res = run_bass_kernel_spmd(nc, in_maps, core_ids=list(range(N_CORES)),
                               **kwargs)
    global _LAST_RES
    _LAST_RES = res
    if trace_dir:
        kernel._last_result = res

    out = np.zeros((B, T, C), dtype=np.float32)
    for c in range(N_CORES):
        out[c // 4] += res.results[c]["out"]
    return out
